# revision 1
# baseline (speedup 1.0000x reference)
"""YOLO DetectionLayer decode kernel for 8 Trainium2 NeuronCores.

Input  x [32, 255, 76, 76] fp32 -> output [32, 17328, 85] fp32.

Key layout fact: per image, out[(hw*3+box)*85 + attr] = f(x[box*85+attr, hw]),
i.e. the output is exactly the transpose of the [255, 5776] channel-major
input with per-channel activations (sigmoid / exp) and an affine box decode.

Per core (4 images): load [255,5776] channel-major, sigmoid in place,
TensorE-transpose 128-col chunks into PSUM, evacuate into a cell-major
SBUF staging tile (overwriting the 12 box-coord columns from a separately
computed "P12" tile holding x1y1 / x2y2 in channel-major), then store
contiguous [cells, 255] rows (= the exact output layout).

Sharding: pure data parallel, batch 32 -> 8 cores x 4 images.
"""
import sys

sys.path.insert(0, '/opt/trn_rl_repo')

import numpy as np

NCORES = 8
BPC = 4          # batch per core
NCH = 255
HW = 5776        # 76*76
NATT = 85
IMG = 608.0
XYS = 1.05
GRID = 76.0
ANCHOR_WH = np.array([[10.0, 13.0], [16.0, 30.0], [33.0, 23.0]], np.float32)

# free-dim halves, aligned to 128-cell chunk boundaries (23 + 22.125 chunks)
HALVES = [(0, 2944), (2944, 2832)]
NCHUNK = 46      # ceil(5776/128); last chunk is 16 cells

_CACHE = {}


def _legalize_waits(nc, mybir):
    """walrus core_v3 rejects >1 wait on most instructions (2 on
    EventSemaphore). Tile's final drain carries one wait per live semaphore;
    split the excess onto preceding EventSemaphore carrier instructions."""
    n_new = 0
    for func in nc.m.functions:
        for block in func.blocks:
            out, changed = [], False
            for inst in block.instructions:
                si = inst.sync_info
                if si is not None:
                    waits = list(si.on_wait or [])
                    cap = 2 if isinstance(inst, mybir.InstEventSemaphore) else 1
                    if len(waits) > cap:
                        keep, extra = waits[:cap], waits[cap:]
                        for i in range(0, len(extra), 2):
                            es = mybir.InstEventSemaphore(
                                name=f"{inst.name}-ws{i}", ins=[], outs=[])
                            es.engine = inst.engine
                            es.sync_info = mybir.SyncInfo(
                                on_wait=list(extra[i:i + 2]), on_update=[])
                            out.append(es)
                            n_new += 1
                        inst.sync_info = mybir.SyncInfo(
                            on_wait=keep, on_update=list(si.on_update or []))
                        changed = True
                out.append(inst)
            if changed:
                block.instructions[:] = out
    return n_new


def make_consts():
    """Host-precomputed constant tensors (identical on every core).

    Row layout of the 128-partition box-coord tiles:
      row = 32*b + dup*6 + box*2 + ch   (b image-in-core, dup 0:x1y1 1:x2y2,
                                         ch 0:x 1:y); rows r%32 >= 12 unused.
    """
    cell = np.arange(HW, dtype=np.float64)
    gx = (cell % 76 - 0.5 * (XYS - 1.0)) / GRID
    gy = (cell // 76 - 0.5 * (XYS - 1.0)) / GRID
    xyoff = np.zeros((128, HW), np.float32)
    anc = np.zeros((128, 1), np.float32)
    for b in range(BPC):
        for dup in range(2):
            for box in range(3):
                for ch in range(2):
                    r = 32 * b + dup * 6 + box * 2 + ch
                    xyoff[r] = (gx if ch == 0 else gy).astype(np.float32)
                    sgn = -1.0 if dup == 0 else 1.0
                    anc[r, 0] = sgn * ANCHOR_WH[box, ch] / (2.0 * IMG)
    return xyoff, anc


def _build(niter=1):
    import concourse.bass as bass
    import concourse.mybir as mybir
    from concourse.tile import TileContext
    from concourse import masks

    F32 = mybir.dt.float32
    AF = mybir.ActivationFunctionType

    nc = bass.Bass("TRN2")
    x = nc.dram_tensor("x", [BPC, NCH, 76, 76], F32, kind="ExternalInput")
    xyoff = nc.dram_tensor("xyoff", [128, HW], F32, kind="ExternalInput")
    anc = nc.dram_tensor("anc", [128, 1], F32, kind="ExternalInput")
    out = nc.dram_tensor("out", [BPC, HW * 3, NATT], F32, kind="ExternalOutput")

    xf = x[:].rearrange("b c h w -> b c (h w)")                  # [4,255,5776]
    xa = xf.rearrange("b (box a) hw -> b box a hw", box=3)       # [4,3,85,5776]
    out2 = out[:].rearrange("b r a -> b (r a)")                  # [4,1473840]

    with TileContext(nc) as tc:
        with tc.tile_pool(name="const", bufs=1) as cpool, \
             tc.tile_pool(name="p12", bufs=min(niter + 1, 2)) as p12pool:
            ident = cpool.tile([128, 128], F32)
            masks.make_identity(nc, ident[:])
            anct = cpool.tile([128, 1], F32)
            nc.scalar.dma_start(out=anct[:], in_=anc[:])

            for it in range(niter):
                # ------------- box-coord precompute (P12), per half -------------
                # p12 row r (layout above) holds, for cells in the half:
                #   dup=0: image_xy - image_wh/2     dup=1: image_xy + image_wh/2
                p12s = []
                with tc.tile_pool(name="tmp", bufs=2) as tmp:
                    for (h0, hw_) in HALVES:
                        xyt = tmp.tile([128, 2944], F32, tag="xyt")
                        wht = tmp.tile([128, 2944], F32, tag="wht")
                        xot = tmp.tile([128, 2944], F32, tag="xot")
                        # prologue loads ride the ACT HWDGE ring so the big
                        # per-image loads on the SP ring start immediately
                        nc.scalar.dma_start(out=xot[:, :hw_],
                                            in_=xyoff[:, h0:h0 + hw_])
                        for b in range(BPC):
                            for dup in range(2):
                                r0 = 32 * b + 6 * dup
                                # dst must stay a plain partition slice: a
                                # rearranged dst lets the AP optimizer merge
                                # partition+free dims, which HW descriptor
                                # generation mislowers (sprays bytes across
                                # neighboring tiles). dma_start only checks
                                # total size, so nested DRAM srcs pair fine.
                                nc.scalar.dma_start(
                                    out=xyt[r0:r0 + 6, :hw_],
                                    in_=xa[b, :, 0:2, h0:h0 + hw_])
                                nc.scalar.dma_start(
                                    out=wht[r0:r0 + 6, :hw_],
                                    in_=xa[b, :, 2:4, h0:h0 + hw_])
                        # image_wh/2 (signed): exp(wh) * (+-anchor/(2*608))
                        nc.scalar.activation(wht[:, :hw_], wht[:, :hw_], AF.Exp)
                        # image_xy: sigmoid(xy)*1.05/76 + (g - 0.025)/76
                        nc.scalar.activation(xyt[:, :hw_], xyt[:, :hw_],
                                             AF.Sigmoid)
                        nc.vector.tensor_scalar_mul(xyt[:, :hw_], xyt[:, :hw_],
                                                    XYS / GRID)
                        nc.vector.tensor_add(xyt[:, :hw_], xyt[:, :hw_],
                                             xot[:, :hw_])
                        nc.vector.tensor_scalar_mul(wht[:, :hw_], wht[:, :hw_],
                                                    anct[:, 0:1])
                        p12 = p12pool.tile([128, 2944], F32, tag=f"p12h")
                        nc.vector.tensor_add(p12[:, :hw_], xyt[:, :hw_],
                                             wht[:, :hw_])
                        p12s.append(p12)

                # ---------------- main per-image pipeline ----------------
                with tc.tile_pool(name="t0", bufs=2) as t0pool, \
                     tc.tile_pool(name="t1", bufs=2) as t1pool, \
                     tc.tile_pool(name="og", bufs=4) as ogpool, \
                     tc.tile_pool(name="ps0", bufs=2, space="PSUM") as ps0pool, \
                     tc.tile_pool(name="ps1", bufs=2, space="PSUM") as ps1pool, \
                     tc.tile_pool(name="psP", bufs=2, space="PSUM") as psPpool:
                    for b in range(BPC):
                        t0h, t1h = [], []
                        for hx, (h0, hw_) in enumerate(HALVES):
                            t0 = t0pool.tile([128, 2944], F32, tag=f"t0{hx}")
                            t1 = t1pool.tile([127, 2944], F32, tag=f"t1{hx}")
                            nc.sync.dma_start(out=t0[:, :hw_],
                                              in_=xf[b, 0:128, h0:h0 + hw_])
                            nc.sync.dma_start(out=t1[:, :hw_],
                                              in_=xf[b, 128:255, h0:h0 + hw_])
                            nc.scalar.activation(t0[:, :hw_], t0[:, :hw_],
                                                 AF.Sigmoid)
                            nc.scalar.activation(t1[:, :hw_], t1[:, :hw_],
                                                 AF.Sigmoid)
                            t0h.append(t0)
                            t1h.append(t1)

                        # b0 leads with a small group so the first store
                        # launches as early as possible (fills the DMA gap
                        # between load-ahead exhaustion and store ramp-up)
                        if b == 0:
                            bounds = [0, 8, 16, 32, 46]
                        elif b == BPC - 1:
                            bounds = [0, 16, 32, 40, 46]
                        else:
                            bounds = [0, 16, 32, 46]
                        for og in range(len(bounds) - 1):
                            j0 = bounds[og]
                            j1 = bounds[og + 1]
                            O = ogpool.tile([128, 4080], F32)
                            for g4 in range(j0, j1, 4):
                                jj = list(range(g4, min(g4 + 4, j1)))
                                n = len(jj)
                                ps0 = ps0pool.tile([128, 512], F32)
                                ps1 = ps1pool.tile([128, 512], F32)
                                psP = psPpool.tile([128, 512], F32)
                                for k, j in enumerate(jj):
                                    c0 = j * 128
                                    w = min(128, HW - c0)
                                    hx = 0 if j < 23 else 1
                                    ch0 = c0 - HALVES[hx][0]
                                    nc.tensor.transpose(
                                        ps0[:w, k * 128:k * 128 + 128],
                                        t0h[hx][:, ch0:ch0 + w], ident[:, :])
                                    nc.tensor.transpose(
                                        ps1[:w, k * 128:k * 128 + 127],
                                        t1h[hx][:, ch0:ch0 + w],
                                        ident[:127, :127])
                                    nc.tensor.transpose(
                                        psP[:w, k * 128:k * 128 + 128],
                                        p12s[hx][:, ch0:ch0 + w], ident[:, :])
                                m = g4 - j0
                                full = all(min(128, HW - j * 128) == 128
                                           for j in jj)
                                if full:
                                    o3 = O[:, m * 255:(m + n) * 255].rearrange(
                                        "p (k a) -> p k a", a=255)
                                    s0 = ps0[:, :n * 128].rearrange(
                                        "p (k a) -> p k a", a=128)
                                    s1 = ps1[:, :n * 128].rearrange(
                                        "p (k a) -> p k a", a=128)
                                    nc.scalar.copy(o3[:, :, 0:128], s0)
                                    nc.vector.tensor_copy(
                                        o3[:, :, 128:255], s1[:, :, 0:127])
                                    dst = O[:, m * 255:(m + n) * 255].rearrange(
                                        "p (k box r) -> p k box r", box=3, r=85
                                    )[:, :, :, 0:4].rearrange(
                                        "p k box (dup ch) -> p k box dup ch",
                                        dup=2)
                                    src = psP[:, :n * 128].rearrange(
                                        "p (k z) -> p k z", z=128
                                    )[:, :, 32 * b:32 * b + 12].rearrange(
                                        "p k (dup box ch) -> p k box dup ch",
                                        dup=2, box=3)
                                    nc.vector.tensor_copy(dst, src)
                                else:
                                    for k, j in enumerate(jj):
                                        w = min(128, HW - j * 128)
                                        ok = O[:, (m + k) * 255:(m + k + 1) * 255]
                                        nc.scalar.copy(
                                            ok[:w, 0:128],
                                            ps0[:w, k * 128:k * 128 + 128])
                                        nc.vector.tensor_copy(
                                            ok[:w, 128:255],
                                            ps1[:w, k * 128:k * 128 + 127])
                                        dst = ok[:w, :].rearrange(
                                            "p (box r) -> p box r", box=3, r=85
                                        )[:, :, 0:4].rearrange(
                                            "p box (dup ch) -> p box dup ch",
                                            dup=2)
                                        src = psP[:w, k * 128 + 32 * b:
                                                  k * 128 + 32 * b + 12].rearrange(
                                            "p (dup box ch) -> p box dup ch",
                                            dup=2, box=3)
                                        nc.vector.tensor_copy(dst, src)
                            # store this output group (ACT HWDGE ring, so the
                            # next image's loads on the SP ring aren't stuck
                            # behind stores in the same FIFO)
                            spans = [(sp, min(sp + 8, j1))
                                     for sp in range(j0, j1, 8)]
                            for (s0, s1) in spans:
                                sf = min(s1, 45)       # full chunks only
                                cell0 = s0 * 128
                                nfull = (sf - s0) * 128
                                m0 = (s0 - j0) * 255
                                dst = out2[b, cell0 * 255:(cell0 + nfull) * 255
                                           ].rearrange("(k p a) -> p k a",
                                                       p=128, a=255)
                                nc.scalar.dma_start(
                                    out=dst,
                                    in_=O[:, m0:m0 + (sf - s0) * 255].rearrange(
                                        "p (k a) -> p k a", a=255))
                                if s1 == NCHUNK:       # 16-cell tail chunk
                                    dst2 = out2[b, 5760 * 255:5776 * 255
                                                ].rearrange("(p a) -> p a", a=255)
                                    nc.scalar.dma_start(
                                        out=dst2,
                                        in_=O[0:16, (45 - j0) * 255:(46 - j0) * 255])

    _legalize_waits(nc, mybir)
    return nc


def _get_built(niter=1):
    if niter not in _CACHE:
        _CACHE[niter] = _build(niter)
    return _CACHE[niter]


def run_on_cores(x, niter=1):
    from concourse import bass_utils
    nc = _get_built(niter)
    xyoff, anc = make_consts()
    x8 = np.ascontiguousarray(np.asarray(x, np.float32).reshape(
        NCORES, BPC, NCH, 76, 76))
    in_maps = [{"x": x8[i], "xyoff": xyoff, "anc": anc}
               for i in range(NCORES)]
    res = bass_utils.run_bass_kernel_spmd(nc, in_maps,
                                          core_ids=list(range(NCORES)))
    outs = np.stack([res.results[i]["out"] for i in range(NCORES)])
    return outs.reshape(NCORES * BPC, HW * 3, NATT)


def kernel(x):
    return run_on_cores(x, niter=1)



# revision 9
# speedup vs baseline: 1.2539x; 1.2539x over previous
"""YOLO DetectionLayer decode kernel for 8 Trainium2 NeuronCores.

Input  x [32, 255, 76, 76] fp32 -> output [32, 17328, 85] fp32.

Key layout fact: per image, out[(hw*3+box)*85 + attr] = f(x[box*85+attr, hw]),
i.e. the output is exactly the transpose of the [255, 5776] channel-major
input with per-channel activations (sigmoid / exp) and an affine box decode.

Per core (4 images): load [255,5776] channel-major (minus the 12 xy/wh
channels, whose output columns are produced separately), sigmoid in place,
TensorE-transpose 128-col chunks into PSUM, evacuate into a cell-major
SBUF staging tile, then store contiguous [cells, 255] rows.

Box coords: one bf16 matmul per 128-cell chunk computes all 12 corner
columns for all 4 images at once:  psP = Rb_chunk.T @ M, where Rb holds
sigmoid(xy) rows (0:48), exp(wh) rows (48:96) and grid-offset rows (96:98),
and the constant M [98,128] bakes in channel selection, x1y1/x2y2
duplication, +-anchor/(2*608) scaling and the grid-offset add.  The
matmul output overwrites the 12 box-coord columns of each output group.

Sharding: pure data parallel, batch 32 -> 8 cores x 4 images.
"""
import sys

sys.path.insert(0, '/opt/trn_rl_repo')

import numpy as np
import ml_dtypes

NCORES = 8
BPC = 4          # batch per core
NCH = 255
HW = 5776        # 76*76
NATT = 85
IMG = 608.0
XYS = 1.05
GRID = 76.0
ANCHOR_WH = np.array([[10.0, 13.0], [16.0, 30.0], [33.0, 23.0]], np.float32)

# free-dim halves, aligned to 128-cell chunk boundaries (23 + 22.125 chunks)
HALVES = [(0, 2944), (2944, 2832)]
NCHUNK = 46      # ceil(5776/128); last chunk is 16 cells

_CACHE = {}


def _legalize_waits(nc, mybir):
    """walrus core_v3 rejects >1 wait on most instructions (2 on
    EventSemaphore). Tile's final drain carries one wait per live semaphore;
    split the excess onto preceding EventSemaphore carrier instructions."""
    n_new = 0
    for func in nc.m.functions:
        for block in func.blocks:
            out, changed = [], False
            for inst in block.instructions:
                si = inst.sync_info
                if si is not None:
                    waits = list(si.on_wait or [])
                    cap = 2 if isinstance(inst, mybir.InstEventSemaphore) else 1
                    if len(waits) > cap:
                        keep, extra = waits[:cap], waits[cap:]
                        for i in range(0, len(extra), 2):
                            es = mybir.InstEventSemaphore(
                                name=f"{inst.name}-ws{i}", ins=[], outs=[])
                            es.engine = inst.engine
                            es.sync_info = mybir.SyncInfo(
                                on_wait=list(extra[i:i + 2]), on_update=[])
                            out.append(es)
                            n_new += 1
                        inst.sync_info = mybir.SyncInfo(
                            on_wait=keep, on_update=list(si.on_update or []))
                        changed = True
                out.append(inst)
            if changed:
                block.instructions[:] = out
    return n_new


def make_consts():
    """Host-precomputed constant tensors (identical on every core).

    mw [114,128] bf16: the box-decode mixing matrix.
      Rb row layout (the matmul K dim): 12*b + box*4 + attr for raw-channel
      sigmoid rows (attr 0..3, only 0:2 used), 64 + same for exp rows (only
      attr 2:4 used; the 64 offset keeps the ACT exp write 32-partition
      aligned), 112+ch for the grid-offset rows.
      psP col layout (matches the evacuate src rearrange):
      j = 32*b + dup*6 + box*2 + ch.
    g [2,HW] bf16: normalized grid offsets (gx-0.025)/76, (gy-0.025)/76.
    """
    cell = np.arange(HW, dtype=np.float64)
    gx = (cell % 76 - 0.5 * (XYS - 1.0)) / GRID
    gy = (cell // 76 - 0.5 * (XYS - 1.0)) / GRID
    g = np.stack([gx, gy]).astype(ml_dtypes.bfloat16)

    mw = np.zeros((114, 128), np.float32)
    for b in range(BPC):
        for dup in range(2):
            for box in range(3):
                for ch in range(2):
                    j = 32 * b + dup * 6 + box * 2 + ch
                    mw[12 * b + box * 4 + ch, j] = XYS / GRID
                    sgn = -1.0 if dup == 0 else 1.0
                    mw[64 + 12 * b + box * 4 + 2 + ch, j] = (
                        sgn * ANCHOR_WH[box, ch] / (2.0 * IMG))
                    mw[112 + ch, j] = 1.0
    mw = mw.astype(ml_dtypes.bfloat16)
    return mw, g


def _build(niter=1):
    import concourse.bass as bass
    import concourse.mybir as mybir
    from concourse.tile import TileContext
    from concourse import masks

    F32 = mybir.dt.float32
    BF16 = mybir.dt.bfloat16
    AF = mybir.ActivationFunctionType

    nc = bass.Bass("TRN2")
    x = nc.dram_tensor("x", [BPC, NCH, 76, 76], F32, kind="ExternalInput")
    mw = nc.dram_tensor("mw", [114, 128], BF16, kind="ExternalInput")
    g = nc.dram_tensor("g", [2, HW], BF16, kind="ExternalInput")
    out = nc.dram_tensor("out", [BPC, HW * 3, NATT], F32, kind="ExternalOutput")

    xf = x[:].rearrange("b c h w -> b c (h w)")                  # [4,255,5776]
    xa = xf.rearrange("b (box a) hw -> b box a hw", box=3)       # [4,3,85,5776]
    out2 = out[:].rearrange("b r a -> b (r a)")                  # [4,1473840]

    with TileContext(nc) as tc:
        with tc.tile_pool(name="const", bufs=1) as cpool, \
             tc.tile_pool(name="rbp", bufs=1) as rbpool:
            ident = cpool.tile([128, 128], F32)
            masks.make_identity(nc, ident[:])
            mt = cpool.tile([114, 128], BF16)
            nc.scalar.dma_start(out=mt[:], in_=mw[:])

            for it in range(niter):
                # --------- box-coord raw loads + activations (per half) -----
                # rr: raw xy/wh channels, 12 rows per image.
                # rb: sigmoid rows 0:48, exp rows 48:96, grid rows 96:98.
                rbs = []
                for (h0, hw_) in HALVES:
                    rr = rbpool.tile([48, 2944], F32, tag="rr")
                    rb = rbpool.tile([114, 2944], BF16, tag="rb")
                    nc.scalar.dma_start(out=rb[112:114, :hw_],
                                        in_=g[:, h0:h0 + hw_])
                    for b in range(BPC):
                        # dst must stay a plain partition slice: a
                        # rearranged dst lets the AP optimizer merge
                        # partition+free dims, which HW descriptor
                        # generation mislowers (sprays bytes across
                        # neighboring tiles). dma_start only checks
                        # total size, so nested DRAM srcs pair fine.
                        nc.scalar.dma_start(
                            out=rr[12 * b:12 * b + 12, :hw_],
                            in_=xa[b, :, 0:4, h0:h0 + hw_])
                    nc.scalar.activation(rb[0:48, :hw_], rr[:, :hw_],
                                         AF.Sigmoid)
                    nc.scalar.activation(rb[64:112, :hw_], rr[:, :hw_],
                                         AF.Exp)
                    rbs.append(rb)

                # ---------------- main per-image pipeline ----------------
                with tc.tile_pool(name="t0", bufs=3) as t0pool, \
                     tc.tile_pool(name="t1", bufs=3) as t1pool, \
                     tc.tile_pool(name="og", bufs=4) as ogpool, \
                     tc.tile_pool(name="ps0", bufs=2, space="PSUM") as ps0pool, \
                     tc.tile_pool(name="ps1", bufs=2, space="PSUM") as ps1pool, \
                     tc.tile_pool(name="psP", bufs=2, space="PSUM") as psPpool:
                    for b in range(BPC):
                        t0h, t1h = [], []
                        for hx, (h0, hw_) in enumerate(HALVES):
                            t0 = t0pool.tile([128, 2944], F32, tag=f"t0{hx}")
                            t1 = t1pool.tile([127, 2944], F32, tag=f"t1{hx}")
                            # skip the xy/wh channels (0:4, 85:89, 170:174):
                            # their output columns are overwritten from psP.
                            nc.sync.dma_start(out=t0[4:85, :hw_],
                                              in_=xf[b, 4:85, h0:h0 + hw_])
                            nc.sync.dma_start(out=t0[89:128, :hw_],
                                              in_=xf[b, 89:128, h0:h0 + hw_])
                            nc.sync.dma_start(out=t1[0:42, :hw_],
                                              in_=xf[b, 128:170, h0:h0 + hw_])
                            nc.sync.dma_start(out=t1[46:127, :hw_],
                                              in_=xf[b, 174:255, h0:h0 + hw_])
                            # full-tile sigmoid: rows 0:4 / 85:89 / 42:46 are
                            # stale (their channels aren't loaded), but their
                            # transposed output columns are overwritten from
                            # psP, so sigmoid(garbage) never reaches out.
                            nc.scalar.activation(t0[:, :hw_], t0[:, :hw_],
                                                 AF.Sigmoid)
                            nc.scalar.activation(t1[:, :hw_], t1[:, :hw_],
                                                 AF.Sigmoid)
                            t0h.append(t0)
                            t1h.append(t1)

                        # uniform 8-chunk output groups: small og tiles (so
                        # more bufs fit in SBUF) and the earliest possible
                        # first store
                        bounds = [0, 8, 16, 24, 32, 40, 46]
                        for og in range(len(bounds) - 1):
                            j0 = bounds[og]
                            j1 = bounds[og + 1]
                            O = ogpool.tile([128, 2040], F32)
                            for g4 in range(j0, j1, 4):
                                jj = list(range(g4, min(g4 + 4, j1)))
                                n = len(jj)
                                ps0 = ps0pool.tile([128, 512], F32)
                                ps1 = ps1pool.tile([128, 512], F32)
                                psP = psPpool.tile([128, 512], F32)
                                for k, j in enumerate(jj):
                                    c0 = j * 128
                                    w = min(128, HW - c0)
                                    hx = 0 if j < 23 else 1
                                    ch0 = c0 - HALVES[hx][0]
                                    nc.tensor.transpose(
                                        ps0[:w, k * 128:k * 128 + 128],
                                        t0h[hx][:, ch0:ch0 + w], ident[:, :])
                                    nc.tensor.transpose(
                                        ps1[:w, k * 128:k * 128 + 127],
                                        t1h[hx][:, ch0:ch0 + w],
                                        ident[:127, :127])
                                    nc.tensor.matmul(
                                        psP[:w, k * 128:k * 128 + 128],
                                        rbs[hx][:, ch0:ch0 + w],
                                        mt[:, :], start=True, stop=True)
                                m = g4 - j0
                                full = all(min(128, HW - j * 128) == 128
                                           for j in jj)
                                if full:
                                    o3 = O[:, m * 255:(m + n) * 255].rearrange(
                                        "p (k a) -> p k a", a=255)
                                    s0 = ps0[:, :n * 128].rearrange(
                                        "p (k a) -> p k a", a=128)
                                    s1 = ps1[:, :n * 128].rearrange(
                                        "p (k a) -> p k a", a=128)
                                    nc.scalar.copy(o3[:, :, 0:128], s0)
                                    nc.vector.tensor_copy(
                                        o3[:, :, 128:255], s1[:, :, 0:127])
                                    dst = O[:, m * 255:(m + n) * 255].rearrange(
                                        "p (k box r) -> p k box r", box=3, r=85
                                    )[:, :, :, 0:4].rearrange(
                                        "p k box (dup ch) -> p k box dup ch",
                                        dup=2)
                                    src = psP[:, :n * 128].rearrange(
                                        "p (k z) -> p k z", z=128
                                    )[:, :, 32 * b:32 * b + 12].rearrange(
                                        "p k (dup box ch) -> p k box dup ch",
                                        dup=2, box=3)
                                    nc.vector.tensor_copy(dst, src)
                                else:
                                    for k, j in enumerate(jj):
                                        w = min(128, HW - j * 128)
                                        ok = O[:, (m + k) * 255:(m + k + 1) * 255]
                                        nc.scalar.copy(
                                            ok[:w, 0:128],
                                            ps0[:w, k * 128:k * 128 + 128])
                                        nc.vector.tensor_copy(
                                            ok[:w, 128:255],
                                            ps1[:w, k * 128:k * 128 + 127])
                                        dst = ok[:w, :].rearrange(
                                            "p (box r) -> p box r", box=3, r=85
                                        )[:, :, 0:4].rearrange(
                                            "p box (dup ch) -> p box dup ch",
                                            dup=2)
                                        src = psP[:w, k * 128 + 32 * b:
                                                  k * 128 + 32 * b + 12].rearrange(
                                            "p (dup box ch) -> p box dup ch",
                                            dup=2, box=3)
                                        nc.vector.tensor_copy(dst, src)
                            # store this output group (ACT HWDGE ring, so the
                            # next image's loads on the SP ring aren't stuck
                            # behind stores in the same FIFO)
                            spans = [(sp, min(sp + 8, j1))
                                     for sp in range(j0, j1, 8)]
                            for (s0, s1) in spans:
                                sf = min(s1, 45)       # full chunks only
                                cell0 = s0 * 128
                                nfull = (sf - s0) * 128
                                m0 = (s0 - j0) * 255
                                dst = out2[b, cell0 * 255:(cell0 + nfull) * 255
                                           ].rearrange("(k p a) -> p k a",
                                                       p=128, a=255)
                                nc.scalar.dma_start(
                                    out=dst,
                                    in_=O[:, m0:m0 + (sf - s0) * 255].rearrange(
                                        "p (k a) -> p k a", a=255))
                                if s1 == NCHUNK:       # 16-cell tail chunk
                                    dst2 = out2[b, 5760 * 255:5776 * 255
                                                ].rearrange("(p a) -> p a", a=255)
                                    nc.scalar.dma_start(
                                        out=dst2,
                                        in_=O[0:16, (45 - j0) * 255:(46 - j0) * 255])

    _legalize_waits(nc, mybir)
    return nc


def _get_built(niter=1):
    if niter not in _CACHE:
        _CACHE[niter] = _build(niter)
    return _CACHE[niter]


def run_on_cores(x, niter=1):
    from concourse import bass_utils
    nc = _get_built(niter)
    mw, g = make_consts()
    x8 = np.ascontiguousarray(np.asarray(x, np.float32).reshape(
        NCORES, BPC, NCH, 76, 76))
    in_maps = [{"x": x8[i], "mw": mw, "g": g}
               for i in range(NCORES)]
    res = bass_utils.run_bass_kernel_spmd(nc, in_maps,
                                          core_ids=list(range(NCORES)))
    outs = np.stack([res.results[i]["out"] for i in range(NCORES)])
    return outs.reshape(NCORES * BPC, HW * 3, NATT)


def kernel(x):
    return run_on_cores(x, niter=1)


# revision 15
# speedup vs baseline: 1.2802x; 1.0210x over previous
"""YOLO DetectionLayer decode kernel for 8 Trainium2 NeuronCores.

Input  x [32, 255, 76, 76] fp32 -> output [32, 17328, 85] fp32.

Key layout fact: per image, out[(hw*3+box)*85 + attr] = f(x[box*85+attr, hw]),
i.e. the output is exactly the transpose of the [255, 5776] channel-major
input with per-channel activations (sigmoid / exp) and an affine box decode.

Per core (4 images): load [255,5776] channel-major (minus the 12 xy/wh
channels, whose output columns are produced separately), sigmoid in place,
TensorE-transpose 128-col chunks into PSUM, evacuate into a cell-major
SBUF staging tile, then store contiguous [cells, 255] rows.

Box coords: one bf16 matmul per 128-cell chunk computes all 12 corner
columns for all 4 images at once:  psP = Rb_chunk.T @ M, where Rb holds
sigmoid(xy) rows (0:48), exp(wh) rows (48:96) and grid-offset rows (96:98),
and the constant M [98,128] bakes in channel selection, x1y1/x2y2
duplication, +-anchor/(2*608) scaling and the grid-offset add.  The
matmul output overwrites the 12 box-coord columns of each output group.

Sharding: pure data parallel, batch 32 -> 8 cores x 4 images.
"""
import sys

sys.path.insert(0, '/opt/trn_rl_repo')

import numpy as np
import ml_dtypes

NCORES = 8
BPC = 4          # batch per core
NCH = 255
HW = 5776        # 76*76
NATT = 85
IMG = 608.0
XYS = 1.05
GRID = 76.0
ANCHOR_WH = np.array([[10.0, 13.0], [16.0, 30.0], [33.0, 23.0]], np.float32)

# free-dim halves, aligned to 128-cell chunk boundaries (23 + 22.125 chunks)
HALVES = [(0, 2944), (2944, 2832)]
NCHUNK = 46      # ceil(5776/128); last chunk is 16 cells

_CACHE = {}


def _legalize_waits(nc, mybir):
    """walrus core_v3 rejects >1 wait on most instructions (2 on
    EventSemaphore). Tile's final drain carries one wait per live semaphore;
    split the excess onto preceding EventSemaphore carrier instructions."""
    n_new = 0
    for func in nc.m.functions:
        for block in func.blocks:
            out, changed = [], False
            for inst in block.instructions:
                si = inst.sync_info
                if si is not None:
                    waits = list(si.on_wait or [])
                    cap = 2 if isinstance(inst, mybir.InstEventSemaphore) else 1
                    if len(waits) > cap:
                        keep, extra = waits[:cap], waits[cap:]
                        for i in range(0, len(extra), 2):
                            es = mybir.InstEventSemaphore(
                                name=f"{inst.name}-ws{i}", ins=[], outs=[])
                            es.engine = inst.engine
                            es.sync_info = mybir.SyncInfo(
                                on_wait=list(extra[i:i + 2]), on_update=[])
                            out.append(es)
                            n_new += 1
                        inst.sync_info = mybir.SyncInfo(
                            on_wait=keep, on_update=list(si.on_update or []))
                        changed = True
                out.append(inst)
            if changed:
                block.instructions[:] = out
    return n_new


def make_consts():
    """Host-precomputed constant tensors (identical on every core).

    mw [114,128] bf16: the box-decode mixing matrix.
      Rb row layout (the matmul K dim): 12*b + box*4 + attr for raw-channel
      sigmoid rows (attr 0..3, only 0:2 used), 64 + same for exp rows (only
      attr 2:4 used; the 64 offset keeps the ACT exp write 32-partition
      aligned), 112+ch for the grid-offset rows.
      psP col layout (matches the evacuate src rearrange):
      j = 32*b + dup*6 + box*2 + ch.
    g [2,HW] bf16: normalized grid offsets (gx-0.025)/76, (gy-0.025)/76.
    """
    cell = np.arange(HW, dtype=np.float64)
    gx = (cell % 76 - 0.5 * (XYS - 1.0)) / GRID
    gy = (cell // 76 - 0.5 * (XYS - 1.0)) / GRID
    g = np.stack([gx, gy]).astype(ml_dtypes.bfloat16)

    mw = np.zeros((114, 128), np.float32)
    for b in range(BPC):
        for dup in range(2):
            for box in range(3):
                for ch in range(2):
                    j = 32 * b + dup * 6 + box * 2 + ch
                    mw[12 * b + box * 4 + ch, j] = XYS / GRID
                    sgn = -1.0 if dup == 0 else 1.0
                    mw[64 + 12 * b + box * 4 + 2 + ch, j] = (
                        sgn * ANCHOR_WH[box, ch] / (2.0 * IMG))
                    mw[112 + ch, j] = 1.0
    mw = mw.astype(ml_dtypes.bfloat16)
    return mw, g


def _build(niter=1):
    import concourse.bass as bass
    import concourse.mybir as mybir
    from concourse.tile import TileContext
    from concourse import masks

    F32 = mybir.dt.float32
    BF16 = mybir.dt.bfloat16
    AF = mybir.ActivationFunctionType

    nc = bass.Bass("TRN2")
    x = nc.dram_tensor("x", [BPC, NCH, 76, 76], F32, kind="ExternalInput")
    mw = nc.dram_tensor("mw", [114, 128], BF16, kind="ExternalInput")
    g = nc.dram_tensor("g", [2, HW], BF16, kind="ExternalInput")
    out = nc.dram_tensor("out", [BPC, HW * 3, NATT], F32, kind="ExternalOutput")

    xf = x[:].rearrange("b c h w -> b c (h w)")                  # [4,255,5776]
    xa = xf.rearrange("b (box a) hw -> b box a hw", box=3)       # [4,3,85,5776]
    out2 = out[:].rearrange("b r a -> b (r a)")                  # [4,1473840]

    with TileContext(nc) as tc:
        with tc.tile_pool(name="const", bufs=1) as cpool, \
             tc.tile_pool(name="rbp", bufs=1) as rbpool:
            ident = cpool.tile([128, 128], F32)
            masks.make_identity(nc, ident[:])
            mt = cpool.tile([114, 128], BF16)
            nc.scalar.dma_start(out=mt[:], in_=mw[:])

            for it in range(niter):
                # --------- box-coord raw loads (per half) -----
                # rr: raw xy/wh channels, 12 rows per image.
                # rb: sigmoid rows 0:48, exp rows 64:112, grid rows 112:114.
                # Loads ride the Pool/SWDGE ring so they don't clog the ACT
                # sequencer (whose HWDGE dispatch contends with SP loads).
                # per-half tags: both halves' rb tiles are live (read by
                # matmuls) for the whole image loop, so they must not share
                # a rotation slot - that creates an in-order PE queue cycle
                # (deadlock).
                rrs, rbs = [], []
                for hx, (h0, hw_) in enumerate(HALVES):
                    rr = rbpool.tile([48, 2944], F32, tag=f"rr{hx}")
                    rb = rbpool.tile([114, 2944], BF16, tag=f"rb{hx}")
                    nc.gpsimd.dma_start(out=rb[112:114, :hw_],
                                        in_=g[:, h0:h0 + hw_])
                    for b in range(BPC):
                        # dst must stay a plain partition slice: a
                        # rearranged dst lets the AP optimizer merge
                        # partition+free dims, which HW descriptor
                        # generation mislowers (sprays bytes across
                        # neighboring tiles). dma_start only checks
                        # total size, so nested DRAM srcs pair fine.
                        nc.gpsimd.dma_start(
                            out=rr[12 * b:12 * b + 12, :hw_],
                            in_=xa[b, :, 0:4, h0:h0 + hw_])
                    rrs.append(rr)
                    rbs.append(rb)

                def emit_rb_acts(hx):
                    hw_ = HALVES[hx][1]
                    nc.scalar.activation(rbs[hx][0:48, :hw_],
                                         rrs[hx][:, :hw_], AF.Sigmoid)
                    nc.scalar.activation(rbs[hx][64:112, :hw_],
                                         rrs[hx][:, :hw_], AF.Exp)

                # rb-h0 activations up front; rb-h1 deferred until image 0's
                # h1 section so the first store chain isn't queued behind
                # them on the in-order ACT sequencer.
                emit_rb_acts(0)

                # chunk-group bounds per half (chunk 23 = first h1 chunk)
                HBOUNDS = [[0, 8, 16, 23], [23, 31, 39, 46]]

                # ---------------- main per-image pipeline ----------------
                with tc.tile_pool(name="t0", bufs=2) as t0pool, \
                     tc.tile_pool(name="t1", bufs=2) as t1pool, \
                     tc.tile_pool(name="og", bufs=4) as ogpool, \
                     tc.tile_pool(name="ps0", bufs=3, space="PSUM") as ps0pool, \
                     tc.tile_pool(name="ps1", bufs=3, space="PSUM") as ps1pool, \
                     tc.tile_pool(name="psP", bufs=2, space="PSUM") as psPpool:
                    for b in range(BPC):
                        for hx, (h0, hw_) in enumerate(HALVES):
                            t0 = t0pool.tile([128, 2944], F32, tag=f"t0{hx}")
                            t1 = t1pool.tile([127, 2944], F32, tag=f"t1{hx}")
                            # skip the xy/wh channels (0:4, 85:89, 170:174):
                            # their output columns are overwritten from psP.
                            nc.sync.dma_start(out=t0[4:85, :hw_],
                                              in_=xf[b, 4:85, h0:h0 + hw_])
                            nc.sync.dma_start(out=t0[89:128, :hw_],
                                              in_=xf[b, 89:128, h0:h0 + hw_])
                            nc.sync.dma_start(out=t1[0:42, :hw_],
                                              in_=xf[b, 128:170, h0:h0 + hw_])
                            nc.sync.dma_start(out=t1[46:127, :hw_],
                                              in_=xf[b, 174:255, h0:h0 + hw_])
                            if b == 0 and hx == 1:
                                emit_rb_acts(1)
                            # full-tile sigmoid: rows 0:4 / 85:89 / 42:46 are
                            # stale (their channels aren't loaded), but their
                            # transposed output columns are overwritten from
                            # psP, so sigmoid(garbage) never reaches out.
                            nc.scalar.activation(t0[:, :hw_], t0[:, :hw_],
                                                 AF.Sigmoid)
                            nc.scalar.activation(t1[:, :hw_], t1[:, :hw_],
                                                 AF.Sigmoid)

                            bounds = HBOUNDS[hx]
                            for og in range(len(bounds) - 1):
                                j0 = bounds[og]
                                j1 = bounds[og + 1]
                                O = ogpool.tile([128, 2040], F32)
                                for g4 in range(j0, j1, 4):
                                    jj = list(range(g4, min(g4 + 4, j1)))
                                    n = len(jj)
                                    ps0 = ps0pool.tile([128, 512], F32)
                                    ps1 = ps1pool.tile([128, 512], F32)
                                    psP = psPpool.tile([128, 512], F32)
                                    for k, j in enumerate(jj):
                                        c0 = j * 128
                                        w = min(128, HW - c0)
                                        ch0 = c0 - h0
                                        nc.tensor.transpose(
                                            ps0[:w, k * 128:k * 128 + 128],
                                            t0[:, ch0:ch0 + w], ident[:, :])
                                        nc.tensor.transpose(
                                            ps1[:w, k * 128:k * 128 + 127],
                                            t1[:, ch0:ch0 + w],
                                            ident[:127, :127])
                                        nc.tensor.matmul(
                                            psP[:w, k * 128:k * 128 + 128],
                                            rbs[hx][:, ch0:ch0 + w],
                                            mt[:, :], start=True, stop=True)
                                    m = g4 - j0
                                    full = all(min(128, HW - j * 128) == 128
                                               for j in jj)
                                    if full:
                                        o3 = O[:, m * 255:(m + n) * 255].rearrange(
                                            "p (k a) -> p k a", a=255)
                                        s0 = ps0[:, :n * 128].rearrange(
                                            "p (k a) -> p k a", a=128)
                                        s1 = ps1[:, :n * 128].rearrange(
                                            "p (k a) -> p k a", a=128)
                                        nc.scalar.copy(o3[:, :, 0:128], s0)
                                        nc.vector.tensor_copy(
                                            o3[:, :, 128:255], s1[:, :, 0:127])
                                        dst = O[:, m * 255:(m + n) * 255].rearrange(
                                            "p (k box r) -> p k box r", box=3, r=85
                                        )[:, :, :, 0:4].rearrange(
                                            "p k box (dup ch) -> p k box dup ch",
                                            dup=2)
                                        src = psP[:, :n * 128].rearrange(
                                            "p (k z) -> p k z", z=128
                                        )[:, :, 32 * b:32 * b + 12].rearrange(
                                            "p k (dup box ch) -> p k box dup ch",
                                            dup=2, box=3)
                                        nc.vector.tensor_copy(dst, src)
                                    else:
                                        for k, j in enumerate(jj):
                                            w = min(128, HW - j * 128)
                                            ok = O[:, (m + k) * 255:(m + k + 1) * 255]
                                            nc.scalar.copy(
                                                ok[:w, 0:128],
                                                ps0[:w, k * 128:k * 128 + 128])
                                            nc.vector.tensor_copy(
                                                ok[:w, 128:255],
                                                ps1[:w, k * 128:k * 128 + 127])
                                            dst = ok[:w, :].rearrange(
                                                "p (box r) -> p box r", box=3, r=85
                                            )[:, :, 0:4].rearrange(
                                                "p box (dup ch) -> p box dup ch",
                                                dup=2)
                                            src = psP[:w, k * 128 + 32 * b:
                                                      k * 128 + 32 * b + 12].rearrange(
                                                "p (dup box ch) -> p box dup ch",
                                                dup=2, box=3)
                                            nc.vector.tensor_copy(dst, src)
                                # store this output group (ACT HWDGE ring, so
                                # the next loads on the SP ring aren't stuck
                                # behind stores in the same FIFO)
                                sf = min(j1, 45)       # full chunks only
                                cell0 = j0 * 128
                                nfull = (sf - j0) * 128
                                dst = out2[b, cell0 * 255:(cell0 + nfull) * 255
                                           ].rearrange("(k p a) -> p k a",
                                                       p=128, a=255)
                                nc.scalar.dma_start(
                                    out=dst,
                                    in_=O[:, :(sf - j0) * 255].rearrange(
                                        "p (k a) -> p k a", a=255))
                                if j1 == NCHUNK:       # 16-cell tail chunk
                                    dst2 = out2[b, 5760 * 255:5776 * 255
                                                ].rearrange("(p a) -> p a", a=255)
                                    nc.scalar.dma_start(
                                        out=dst2,
                                        in_=O[0:16, (45 - j0) * 255:(46 - j0) * 255])

    _legalize_waits(nc, mybir)
    return nc


def _get_built(niter=1):
    if niter not in _CACHE:
        _CACHE[niter] = _build(niter)
    return _CACHE[niter]


def run_on_cores(x, niter=1):
    from concourse import bass_utils
    nc = _get_built(niter)
    mw, g = make_consts()
    x8 = np.ascontiguousarray(np.asarray(x, np.float32).reshape(
        NCORES, BPC, NCH, 76, 76))
    in_maps = [{"x": x8[i], "mw": mw, "g": g}
               for i in range(NCORES)]
    res = bass_utils.run_bass_kernel_spmd(nc, in_maps,
                                          core_ids=list(range(NCORES)))
    outs = np.stack([res.results[i]["out"] for i in range(NCORES)])
    return outs.reshape(NCORES * BPC, HW * 3, NATT)


def kernel(x):
    return run_on_cores(x, niter=1)


# revision 17
# speedup vs baseline: 1.2924x; 1.0095x over previous
"""YOLO DetectionLayer decode kernel for 8 Trainium2 NeuronCores.

Input  x [32, 255, 76, 76] fp32 -> output [32, 17328, 85] fp32.

Key layout fact: per image, out[(hw*3+box)*85 + attr] = f(x[box*85+attr, hw]),
i.e. the output is exactly the transpose of the [255, 5776] channel-major
input with per-channel activations (sigmoid / exp) and an affine box decode.

Per core (4 images): load [255,5776] channel-major (minus the 12 xy/wh
channels, whose output columns are produced separately), sigmoid in place,
TensorE-transpose 128-col chunks into PSUM, evacuate into a cell-major
SBUF staging tile, then store contiguous [cells, 255] rows.

Box coords: one bf16 matmul per 128-cell chunk computes all 12 corner
columns for all 4 images at once:  psP = Rb_chunk.T @ M, where Rb holds
sigmoid(xy) rows (0:48), exp(wh) rows (48:96) and grid-offset rows (96:98),
and the constant M [98,128] bakes in channel selection, x1y1/x2y2
duplication, +-anchor/(2*608) scaling and the grid-offset add.  The
matmul output overwrites the 12 box-coord columns of each output group.

Sharding: pure data parallel, batch 32 -> 8 cores x 4 images.
"""
import sys

sys.path.insert(0, '/opt/trn_rl_repo')

import numpy as np
import ml_dtypes

NCORES = 8
BPC = 4          # batch per core
NCH = 255
HW = 5776        # 76*76
NATT = 85
IMG = 608.0
XYS = 1.05
GRID = 76.0
ANCHOR_WH = np.array([[10.0, 13.0], [16.0, 30.0], [33.0, 23.0]], np.float32)

# free-dim halves, aligned to 128-cell chunk boundaries (23 + 22.125 chunks)
HALVES = [(0, 2944), (2944, 2832)]
NCHUNK = 46      # ceil(5776/128); last chunk is 16 cells

_CACHE = {}


def _legalize_waits(nc, mybir):
    """walrus core_v3 rejects >1 wait on most instructions (2 on
    EventSemaphore). Tile's final drain carries one wait per live semaphore;
    split the excess onto preceding EventSemaphore carrier instructions."""
    n_new = 0
    for func in nc.m.functions:
        for block in func.blocks:
            out, changed = [], False
            for inst in block.instructions:
                si = inst.sync_info
                if si is not None:
                    waits = list(si.on_wait or [])
                    cap = 2 if isinstance(inst, mybir.InstEventSemaphore) else 1
                    if len(waits) > cap:
                        keep, extra = waits[:cap], waits[cap:]
                        for i in range(0, len(extra), 2):
                            es = mybir.InstEventSemaphore(
                                name=f"{inst.name}-ws{i}", ins=[], outs=[])
                            es.engine = inst.engine
                            es.sync_info = mybir.SyncInfo(
                                on_wait=list(extra[i:i + 2]), on_update=[])
                            out.append(es)
                            n_new += 1
                        inst.sync_info = mybir.SyncInfo(
                            on_wait=keep, on_update=list(si.on_update or []))
                        changed = True
                out.append(inst)
            if changed:
                block.instructions[:] = out
    return n_new


def make_consts():
    """Host-precomputed constant tensors (identical on every core).

    mw [114,128] bf16: the box-decode mixing matrix.
      Rb row layout (the matmul K dim): 12*b + box*4 + attr for raw-channel
      sigmoid rows (attr 0..3, only 0:2 used), 64 + same for exp rows (only
      attr 2:4 used; the 64 offset keeps the ACT exp write 32-partition
      aligned), 112+ch for the grid-offset rows.
      psP col layout (matches the evacuate src rearrange):
      j = 32*b + dup*6 + box*2 + ch.
    g [2,HW] bf16: normalized grid offsets (gx-0.025)/76, (gy-0.025)/76.
    """
    cell = np.arange(HW, dtype=np.float64)
    gx = (cell % 76 - 0.5 * (XYS - 1.0)) / GRID
    gy = (cell // 76 - 0.5 * (XYS - 1.0)) / GRID
    g = np.stack([gx, gy]).astype(ml_dtypes.bfloat16)

    mw = np.zeros((114, 128), np.float32)
    for b in range(BPC):
        for dup in range(2):
            for box in range(3):
                for ch in range(2):
                    j = 32 * b + dup * 6 + box * 2 + ch
                    mw[12 * b + box * 4 + ch, j] = XYS / GRID
                    sgn = -1.0 if dup == 0 else 1.0
                    mw[64 + 12 * b + box * 4 + 2 + ch, j] = (
                        sgn * ANCHOR_WH[box, ch] / (2.0 * IMG))
                    mw[112 + ch, j] = 1.0
    mw = mw.astype(ml_dtypes.bfloat16)
    return mw, g


def _build(niter=1):
    import concourse.bass as bass
    import concourse.mybir as mybir
    from concourse.tile import TileContext
    from concourse import masks

    F32 = mybir.dt.float32
    BF16 = mybir.dt.bfloat16
    AF = mybir.ActivationFunctionType

    nc = bass.Bass("TRN2")
    x = nc.dram_tensor("x", [BPC, NCH, 76, 76], F32, kind="ExternalInput")
    mw = nc.dram_tensor("mw", [114, 128], BF16, kind="ExternalInput")
    g = nc.dram_tensor("g", [2, HW], BF16, kind="ExternalInput")
    out = nc.dram_tensor("out", [BPC, HW * 3, NATT], F32, kind="ExternalOutput")

    xf = x[:].rearrange("b c h w -> b c (h w)")                  # [4,255,5776]
    xa = xf.rearrange("b (box a) hw -> b box a hw", box=3)       # [4,3,85,5776]
    out2 = out[:].rearrange("b r a -> b (r a)")                  # [4,1473840]

    with TileContext(nc) as tc:
        with tc.tile_pool(name="const", bufs=1) as cpool, \
             tc.tile_pool(name="rbp", bufs=1) as rbpool:
            ident = cpool.tile([128, 128], F32)
            masks.make_identity(nc, ident[:])
            mt = cpool.tile([114, 128], BF16)
            nc.scalar.dma_start(out=mt[:], in_=mw[:])

            for it in range(niter):
                # --------- box-coord raw loads (per half) -----
                # rr: raw xy/wh channels, 12 rows per image.
                # rb: sigmoid rows 0:48, exp rows 64:112, grid rows 112:114.
                # Loads ride the Pool/SWDGE ring so they don't clog the ACT
                # sequencer (whose HWDGE dispatch contends with SP loads).
                # per-half tags: both halves' rb tiles are live (read by
                # matmuls) for the whole image loop, so they must not share
                # a rotation slot - that creates an in-order PE queue cycle
                # (deadlock).
                rrs, rbs = [], []
                for hx, (h0, hw_) in enumerate(HALVES):
                    rr = rbpool.tile([48, 2944], F32, tag=f"rr{hx}")
                    rb = rbpool.tile([114, 2944], BF16, tag=f"rb{hx}")
                    nc.gpsimd.dma_start(out=rb[112:114, :hw_],
                                        in_=g[:, h0:h0 + hw_])
                    for b in range(BPC):
                        # dst must stay a plain partition slice: a
                        # rearranged dst lets the AP optimizer merge
                        # partition+free dims, which HW descriptor
                        # generation mislowers (sprays bytes across
                        # neighboring tiles). dma_start only checks
                        # total size, so nested DRAM srcs pair fine.
                        nc.gpsimd.dma_start(
                            out=rr[12 * b:12 * b + 12, :hw_],
                            in_=xa[b, :, 0:4, h0:h0 + hw_])
                    rrs.append(rr)
                    rbs.append(rb)

                def emit_rb_acts(hx):
                    hw_ = HALVES[hx][1]
                    nc.scalar.activation(rbs[hx][0:48, :hw_],
                                         rrs[hx][:, :hw_], AF.Sigmoid)
                    nc.scalar.activation(rbs[hx][64:112, :hw_],
                                         rrs[hx][:, :hw_], AF.Exp)

                # rb-h0 activations up front; rb-h1 deferred until image 0's
                # h1 section so the first store chain isn't queued behind
                # them on the in-order ACT sequencer.
                emit_rb_acts(0)

                # chunk-group bounds per half (chunk 23 = first h1 chunk)
                HBOUNDS = [[0, 8, 16, 23], [23, 31, 39, 46]]

                # ---------------- main per-image pipeline ----------------
                with tc.tile_pool(name="t0", bufs=2) as t0pool, \
                     tc.tile_pool(name="t1", bufs=2) as t1pool, \
                     tc.tile_pool(name="og", bufs=4) as ogpool, \
                     tc.tile_pool(name="ps0", bufs=2, space="PSUM") as ps0pool, \
                     tc.tile_pool(name="ps1", bufs=2, space="PSUM") as ps1pool, \
                     tc.tile_pool(name="psP", bufs=2, space="PSUM") as psPpool:
                    for b in range(BPC):
                        for hx, (h0, hw_) in enumerate(HALVES):
                            t0 = t0pool.tile([128, 2944], F32, tag=f"t0{hx}")
                            t1 = t1pool.tile([127, 2944], F32, tag=f"t1{hx}")
                            # skip the xy/wh channels (0:4, 85:89, 170:174):
                            # their output columns are overwritten from psP.
                            nc.sync.dma_start(out=t0[4:85, :hw_],
                                              in_=xf[b, 4:85, h0:h0 + hw_])
                            nc.sync.dma_start(out=t0[89:128, :hw_],
                                              in_=xf[b, 89:128, h0:h0 + hw_])
                            nc.sync.dma_start(out=t1[0:42, :hw_],
                                              in_=xf[b, 128:170, h0:h0 + hw_])
                            nc.sync.dma_start(out=t1[46:127, :hw_],
                                              in_=xf[b, 174:255, h0:h0 + hw_])
                            if b == 0 and hx == 1:
                                emit_rb_acts(1)

                            bounds = HBOUNDS[hx]
                            for og in range(len(bounds) - 1):
                                j0 = bounds[og]
                                j1 = bounds[og + 1]
                                # per-group sigmoid column slice: the group's
                                # transposes wait only on their own cells, not
                                # the whole half. Full-tile rows: 0:4 / 85:89
                                # / 42:46 are stale (channels not loaded), but
                                # their transposed output columns are
                                # overwritten from psP, so sigmoid(garbage)
                                # never reaches out.
                                sc0 = j0 * 128 - h0
                                sc1 = min(j1 * 128, HW) - h0
                                nc.scalar.activation(t0[:, sc0:sc1],
                                                     t0[:, sc0:sc1],
                                                     AF.Sigmoid)
                                nc.scalar.activation(t1[:, sc0:sc1],
                                                     t1[:, sc0:sc1],
                                                     AF.Sigmoid)
                                O = ogpool.tile([128, 2040], F32)
                                for g4 in range(j0, j1, 4):
                                    jj = list(range(g4, min(g4 + 4, j1)))
                                    n = len(jj)
                                    ps0 = ps0pool.tile([128, 512], F32)
                                    ps1 = ps1pool.tile([128, 512], F32)
                                    psP = psPpool.tile([128, 512], F32)
                                    for k, j in enumerate(jj):
                                        c0 = j * 128
                                        w = min(128, HW - c0)
                                        ch0 = c0 - h0
                                        nc.tensor.transpose(
                                            ps0[:w, k * 128:k * 128 + 128],
                                            t0[:, ch0:ch0 + w], ident[:, :])
                                        nc.tensor.transpose(
                                            ps1[:w, k * 128:k * 128 + 127],
                                            t1[:, ch0:ch0 + w],
                                            ident[:127, :127])
                                        nc.tensor.matmul(
                                            psP[:w, k * 128:k * 128 + 128],
                                            rbs[hx][:, ch0:ch0 + w],
                                            mt[:, :], start=True, stop=True)
                                    m = g4 - j0
                                    full = all(min(128, HW - j * 128) == 128
                                               for j in jj)
                                    if full:
                                        o3 = O[:, m * 255:(m + n) * 255].rearrange(
                                            "p (k a) -> p k a", a=255)
                                        s0 = ps0[:, :n * 128].rearrange(
                                            "p (k a) -> p k a", a=128)
                                        s1 = ps1[:, :n * 128].rearrange(
                                            "p (k a) -> p k a", a=128)
                                        nc.scalar.copy(o3[:, :, 0:128], s0)
                                        nc.vector.tensor_copy(
                                            o3[:, :, 128:255], s1[:, :, 0:127])
                                        dst = O[:, m * 255:(m + n) * 255].rearrange(
                                            "p (k box r) -> p k box r", box=3, r=85
                                        )[:, :, :, 0:4].rearrange(
                                            "p k box (dup ch) -> p k box dup ch",
                                            dup=2)
                                        src = psP[:, :n * 128].rearrange(
                                            "p (k z) -> p k z", z=128
                                        )[:, :, 32 * b:32 * b + 12].rearrange(
                                            "p k (dup box ch) -> p k box dup ch",
                                            dup=2, box=3)
                                        nc.vector.tensor_copy(dst, src)
                                    else:
                                        for k, j in enumerate(jj):
                                            w = min(128, HW - j * 128)
                                            ok = O[:, (m + k) * 255:(m + k + 1) * 255]
                                            nc.scalar.copy(
                                                ok[:w, 0:128],
                                                ps0[:w, k * 128:k * 128 + 128])
                                            nc.vector.tensor_copy(
                                                ok[:w, 128:255],
                                                ps1[:w, k * 128:k * 128 + 127])
                                            dst = ok[:w, :].rearrange(
                                                "p (box r) -> p box r", box=3, r=85
                                            )[:, :, 0:4].rearrange(
                                                "p box (dup ch) -> p box dup ch",
                                                dup=2)
                                            src = psP[:w, k * 128 + 32 * b:
                                                      k * 128 + 32 * b + 12].rearrange(
                                                "p (dup box ch) -> p box dup ch",
                                                dup=2, box=3)
                                            nc.vector.tensor_copy(dst, src)
                                # store this output group (ACT HWDGE ring, so
                                # the next loads on the SP ring aren't stuck
                                # behind stores in the same FIFO). The very
                                # last group stores per-g4 so the final DMA
                                # transfer (gating kernel end) is small.
                                last_group = (b == BPC - 1 and j1 == NCHUNK)
                                spans = ([(sp, min(sp + 4, j1))
                                          for sp in range(j0, j1, 4)]
                                         if last_group else [(j0, j1)])
                                for (sp0, sp1) in spans:
                                    sf = min(sp1, 45)  # full chunks only
                                    cell0 = sp0 * 128
                                    nfull = (sf - sp0) * 128
                                    m0 = (sp0 - j0) * 255
                                    dst = out2[b, cell0 * 255:
                                               (cell0 + nfull) * 255
                                               ].rearrange("(k p a) -> p k a",
                                                           p=128, a=255)
                                    nc.scalar.dma_start(
                                        out=dst,
                                        in_=O[:, m0:m0 + (sf - sp0) * 255
                                              ].rearrange("p (k a) -> p k a",
                                                          a=255))
                                    if sp1 == NCHUNK:  # 16-cell tail chunk
                                        dst2 = out2[b, 5760 * 255:5776 * 255
                                                    ].rearrange("(p a) -> p a",
                                                                a=255)
                                        nc.scalar.dma_start(
                                            out=dst2,
                                            in_=O[0:16, (45 - j0) * 255:
                                                  (46 - j0) * 255])

    _legalize_waits(nc, mybir)
    return nc


def _get_built(niter=1):
    if niter not in _CACHE:
        _CACHE[niter] = _build(niter)
    return _CACHE[niter]


def run_on_cores(x, niter=1):
    from concourse import bass_utils
    nc = _get_built(niter)
    mw, g = make_consts()
    x8 = np.ascontiguousarray(np.asarray(x, np.float32).reshape(
        NCORES, BPC, NCH, 76, 76))
    in_maps = [{"x": x8[i], "mw": mw, "g": g}
               for i in range(NCORES)]
    res = bass_utils.run_bass_kernel_spmd(nc, in_maps,
                                          core_ids=list(range(NCORES)))
    outs = np.stack([res.results[i]["out"] for i in range(NCORES)])
    return outs.reshape(NCORES * BPC, HW * 3, NATT)


def kernel(x):
    return run_on_cores(x, niter=1)
